# revision 3
# baseline (speedup 1.0000x reference)
"""Soft-DTW loss (gamma=0.1) for pred/target [64, 512] f32 on 8 Trainium2 cores.

Bidirectional decomposition: soft-DTW factorizes exactly at a row cut —
  loss = -g*ln( sum_j e^{-(F[M,j]+B[M+1,j])/g} + sum_j e^{-(F[M,j]+B[M+1,j+1])/g} )
with F the forward DP and B the backward DP (= forward DP on reversed
sequences).  Cores 0-3 run the forward half (rows 1..256), cores 4-7 run the
backward half on reversed inputs — all 8 cores execute the SAME program, each
on 16 batch elements, 256 DP rows.  The host combines the two boundary rows
per batch (logsumexp) and averages.

Per-core DP: banded (band cols lo(r)=max(r-47,0) .. lo(r)+95, width 96) with a
hard min-plus scaffold scan + exact linear-scan soft correction per row:
  scan1:  Sh[j] = min(m2[j], Sh[j-1]) + d[j]          (min-plus scaffold)
  scan2:  E[j]  = A[j]*E[j-1] + C1[j]*Ep[.] + C2[j]*Ep[.]   (E = e^{(Sh-R)/g})
Every 64 rows Sh is re-anchored (Sh -= g*ln E, E := 1).  Engine split:
Pool: m2/min3/arg0, DVE: scans/argpair/bcat/bsum, Act: one 3*WW-wide exp.
The E-stream is emitted one row behind the scaffold stream so neither
engine stalls on the other's row.
"""
import sys
import numpy as np

sys.path.insert(0, "/opt/trn_rl_repo")

B, L = 64, 512
NCORES = 8
NFWD = 4                    # cores 0..3 forward, 4..7 backward
BLOC = B // NFWD            # 16 batch elements per core
NR = 256                    # DP rows per core (half of L)
W = 47                      # band: cols max(r-47,0) .. +95  (width 96)
WW = 96
NTC = 304                   # target cols staged: lo(255)+WW = 208+96
K_ANCHOR = 64
BIG = 1e30
GAMMA = 0.1
RG = 8                      # rows per dsq ring DMA group
NGRP = NR // RG

LOS = [max(r - W, 0) for r in range(NR)]
SHIFTS = [0] + [LOS[r] - LOS[r - 1] for r in range(1, NR)]
IS_ANCHOR = [((r + 1) % K_ANCHOR == 0) for r in range(NR)]


def _build(trace_sim=False):
    import concourse.bass as bass
    import concourse.tile as tile
    from concourse import bacc, mybir
    import bass_rust

    f32 = mybir.dt.float32
    bf16 = mybir.dt.bfloat16
    fp16 = mybir.dt.float16
    Alu = mybir.AluOpType
    Act = mybir.ActivationFunctionType

    nc = bacc.Bacc("TRN2", target_bir_lowering=False, debug=False,
                   num_devices=NCORES)
    pred_d = nc.dram_tensor("pred", [BLOC, L], f32, kind="ExternalInput").ap()
    target_d = nc.dram_tensor("target", [BLOC, L], f32, kind="ExternalInput").ap()
    out_d = nc.dram_tensor("out", [BLOC, WW], f32, kind="ExternalOutput").ap()
    dsq_d = nc.dram_tensor("dsq_scratch", [BLOC, NR * NTC], fp16,
                           kind="Internal").ap()

    def dram_ap(base_ap, offset, dims):
        return bass_rust.AP(base_ap.tensor, offset, dims)

    def pair_view(t_slice, n, h=2):
        # overlapping pair view: element (h, j) = t_slice[j + h]
        return bass_rust.AP(t_slice.tensor, t_slice.offset,
                            [list(t_slice.ap[0]), [1, h], [1, n]])

    def bcast_view(t_slice, n, h=2):
        # stride-0 broadcast over h: element (h, j) = t_slice[j]
        return bass_rust.AP(t_slice.tensor, t_slice.offset,
                            [list(t_slice.ap[0]), [0, h], [1, n]])

    with tile.TileContext(nc, trace_sim=trace_sim) as tc:
        from contextlib import ExitStack
        with ExitStack() as ctx:
            pre = ctx.enter_context(tc.tile_pool(name="pre", bufs=3))
            tbp = ctx.enter_context(tc.tile_pool(name="tbp", bufs=2))
            per = ctx.enter_context(tc.tile_pool(name="per", bufs=1))
            ring = ctx.enter_context(tc.tile_pool(name="ring", bufs=1))
            rowp = ctx.enter_context(tc.tile_pool(name="rowp", bufs=3))

            # ---------- Phase 1: dsq[b, r, c] = (pred[b,r]-target[b,c])^2
            # staged to DRAM in fp16, rows 0..255 x cols 0..303.
            for b in range(BLOC):
                tb = tbp.tile([128, NTC], f32, tag="tb")
                nc.sync.dma_start(tb[:], dram_ap(target_d, b * L,
                                                 [[0, 128], [1, NTC]]))
                for c in range(2):
                    pcol = pre.tile([128, 1], f32, tag="pcol")
                    nc.sync.dma_start(pcol[:], dram_ap(pred_d, b * L + c * 128,
                                                       [[1, 128], [1, 1]]))
                    sqt = pre.tile([128, NTC], fp16, tag="sqt")
                    if (2 * b + c) % 2 == 0:
                        dtile = pre.tile([128, NTC], f32, tag="dtile")
                        nc.vector.tensor_scalar(dtile[:], tb[:], pcol[:, 0:1],
                                                None, op0=Alu.subtract)
                        nc.vector.scalar_tensor_tensor(
                            sqt[:], dtile[:], 0.0, dtile[:],
                            op0=Alu.bypass, op1=Alu.mult)
                    else:
                        npc = pre.tile([128, 1], f32, tag="npc")
                        nc.gpsimd.tensor_scalar(npc[:], pcol[:], -1.0, None,
                                                op0=Alu.mult)
                        nc.scalar.activation(sqt[:], tb[:], Act.Square,
                                             bias=npc[:, 0:1], scale=1.0)
                    nc.sync.dma_start(
                        dram_ap(dsq_d, b * NR * NTC + c * 128 * NTC,
                                [[NTC, 128], [1, NTC]]), sqt[:])

            # ---------- persistent state tiles
            sh = [per.tile([BLOC, WW + 2], f32, tag=f"sh{i}", name=f"sh{i}")
                  for i in range(2)]
            ee = [per.tile([BLOC, WW + 2], bf16, tag=f"ee{i}", name=f"ee{i}")
                  for i in range(2)]
            sh_init = per.tile([BLOC, WW + 2], f32, tag="shi", name="shi")
            ee_init = per.tile([BLOC, WW + 2], bf16, tag="eei", name="eei")
            for t in sh:
                nc.gpsimd.memset(t[:], BIG)
            for t in ee:
                nc.gpsimd.memset(t[:], 0.0)
            nc.gpsimd.memset(sh_init[:], BIG)
            nc.gpsimd.memset(sh_init[:, 0:1], 0.0)
            nc.gpsimd.memset(ee_init[:], 1.0)

            rings = [ring.tile([BLOC, RG * WW], fp16, tag=f"ring{i}",
                               name=f"ringt{i}") for i in range(4)]

            tc.strict_bb_all_engine_barrier()

            def ring_dma(g):
                r0 = g * RG
                if r0 < 48:                # segment A: lo = 0
                    off, rstep = r0 * NTC, NTC
                else:                      # segment B: lo = r - 47
                    off, rstep = r0 * NTC + (r0 - W), NTC + 1
                src = dram_ap(dsq_d, off,
                              [[NR * NTC, BLOC], [rstep, RG], [1, WW]])
                nc.sync.dma_start(rings[g % 4][:], src)

            for g in range(3):
                ring_dma(g)

            # ---------- Phase 2: main DP loop (E-stream one row behind)
            # Engine split -- DVE: m2/scan1/argpair/bcat/scan2,
            # Pool: min3/arg0/bsum, Act: exp.  Pool has no min/max ALU.
            m2_t = {}     # m2 tile per row, emitted one row early
            e_st = {}     # per-row tiles the skewed E-stream needs

            def emit_m2(r):
                shp = sh_init if r == 0 else sh[(r + 1) % 2]
                s = SHIFTS[r]
                m2 = rowp.tile([BLOC, WW], f32, tag=f"m2{r % 3}")
                nc.vector.scalar_tensor_tensor(
                    m2[:], shp[:, s:s + WW], 0.0, shp[:, s + 1:s + 1 + WW],
                    op0=Alu.bypass, op1=Alu.min)
                m2_t[r] = m2

            def emit_scan1(r):
                g = r // RG
                dsq_row = rings[g % 4][:, (r % RG) * WW:(r % RG + 1) * WW]
                shc = sh[r % 2]
                nc.vector.tensor_tensor_scan(
                    shc[:, 1:WW + 1], m2_t.pop(r)[:], dsq_row, BIG,
                    op0=Alu.min, op1=Alu.add)
                return dsq_row

            def emit_min3(r, dsq_row):
                shc = sh[r % 2]
                min3 = rowp.tile([BLOC, WW], f32, tag=f"min3{r % 3}")
                nc.gpsimd.tensor_tensor(min3[:], shc[:, 1:WW + 1], dsq_row,
                                        op=Alu.subtract)
                return min3

            def emit_arg0(r, min3, argcat):
                shc = sh[r % 2]
                nc.gpsimd.tensor_tensor(argcat[:, 0:WW], min3[:],
                                        shc[:, 0:WW], op=Alu.subtract)

            def emit_argpair(r, min3, argcat):
                shp = sh_init if r == 0 else sh[(r + 1) % 2]
                s = SHIFTS[r]
                nc.vector.scalar_tensor_tensor(
                    argcat[:, WW:3 * WW].rearrange("p (h w) -> p h w", h=2),
                    bcast_view(min3[:], WW), 0.0,
                    pair_view(shp[:, s:s + WW + 1], WW),
                    op0=Alu.bypass, op1=Alu.subtract)

            def emit_exp(r, argcat):
                eacc = rowp.tile([BLOC, 3 * WW], bf16, tag=f"eacc{r % 3}")
                nc.scalar.activation(eacc[:], argcat[:], Act.Exp,
                                     scale=1.0 / GAMMA)
                e_st[r] = eacc

            def emit_bcat(r):
                eacc = e_st[r]
                eep = ee_init if r == 0 else ee[(r + 1) % 2]
                s = SHIFTS[r]
                bcat = rowp.tile([BLOC, 2 * WW], bf16, tag=f"bcat{r % 2}")
                nc.vector.scalar_tensor_tensor(
                    bcat[:].rearrange("p (h w) -> p h w", h=2),
                    eacc[:, WW:3 * WW].rearrange("p (h w) -> p h w", h=2), 0.0,
                    pair_view(eep[:, s:s + WW + 1], WW),
                    op0=Alu.bypass, op1=Alu.mult)
                return bcat

            def emit_bsum(r, bcat):
                bsum = rowp.tile([BLOC, WW], bf16, tag=f"bsum{r % 2}")
                nc.gpsimd.tensor_tensor(bsum[:], bcat[:, 0:WW],
                                        bcat[:, WW:2 * WW], op=Alu.add)
                return bsum

            def emit_scan2(r, bsum):
                eacc = e_st.pop(r)
                eec = ee[r % 2]
                nc.vector.tensor_tensor_scan(
                    eec[:, 1:WW + 1], eacc[:, 0:WW], bsum[:], 0.0,
                    op0=Alu.mult, op1=Alu.add)
                if IS_ANCHOR[r]:
                    shc = sh[r % 2]
                    lne = rowp.tile([BLOC, WW], f32, tag="lne")
                    nc.scalar.activation(lne[:], eec[:, 1:WW + 1], Act.Ln)
                    nc.vector.scalar_tensor_tensor(
                        shc[:, 1:WW + 1], lne[:], -GAMMA, shc[:, 1:WW + 1],
                        op0=Alu.mult, op1=Alu.add)
                    nc.gpsimd.memset(eec[:, 1:WW + 1], 1.0)

            def emit_e(r):
                emit_scan2(r, emit_bsum(r, emit_bcat(r)))

            emit_m2(0)
            for r in range(NR):
                g = r // RG
                if r % RG == 0 and g + 3 < NGRP:
                    ring_dma(g + 3)
                if r >= 1 and IS_ANCHOR[r - 1]:
                    emit_e(r - 1)          # catch up + anchor BEFORE advancing
                dsq_row = emit_scan1(r)
                min3 = emit_min3(r, dsq_row)
                bc = bs = None
                if r >= 1 and not IS_ANCHOR[r - 1]:
                    bc = emit_bcat(r - 1)
                argcat = rowp.tile([BLOC, 3 * WW], f32, tag=f"argcat{r % 3}")
                emit_arg0(r, min3, argcat)
                if r + 1 < NR:
                    emit_m2(r + 1)
                if bc is not None:
                    bs = emit_bsum(r - 1, bc)
                emit_argpair(r, min3, argcat)
                emit_exp(r, argcat)
                if bs is not None:
                    emit_scan2(r - 1, bs)
            emit_e(NR - 1)

            nc.sync.dma_start(out_d[:, :], sh[(NR - 1) % 2][:, 1:WW + 1])

    nc.compile()
    return nc


_NC = None


def _make_in_maps(pred, target):
    pred = np.ascontiguousarray(pred, dtype=np.float32)
    target = np.ascontiguousarray(target, dtype=np.float32)
    predr = np.ascontiguousarray(pred[:, ::-1])
    targetr = np.ascontiguousarray(target[:, ::-1])
    in_maps = []
    for m in range(NFWD):
        in_maps.append({"pred": pred[m * BLOC:(m + 1) * BLOC],
                        "target": target[m * BLOC:(m + 1) * BLOC]})
    for m in range(NFWD):
        in_maps.append({"pred": predr[m * BLOC:(m + 1) * BLOC],
                        "target": targetr[m * BLOC:(m + 1) * BLOC]})
    return in_maps


def _combine(fwd_rows, bwd_rows):
    """fwd_rows/bwd_rows: [B, WW] f64. F[256, 209+jb] and F'[256, 209+jb].
    B[257, j] = F'[256, 513-j]  ->  j = 513 - (209+jb) = 304 - jb."""
    g = GAMMA
    lo1 = LOS[NR - 1] + 1                      # 209: first DP col of F row
    Ffull = np.full((B, L + 2), np.inf)
    Bfull = np.full((B, L + 2), np.inf)
    cols = lo1 + np.arange(WW)                 # 209..304
    Ffull[:, cols] = fwd_rows
    Bfull[:, L + 1 - cols] = bwd_rows          # 304..209
    t1 = Ffull[:, 1:L + 1] + Bfull[:, 1:L + 1]
    t2 = Ffull[:, 1:L + 1] + Bfull[:, 2:L + 2]
    allt = np.concatenate([t1, t2], axis=1)
    m = np.minimum(allt.min(axis=1), BIG)
    ex = np.exp(-(allt - m[:, None]) / g)
    return m - g * np.log(ex.sum(axis=1))


def kernel(pred: np.ndarray, target: np.ndarray) -> np.ndarray:
    global _NC
    from concourse.bass_utils import run_bass_kernel_spmd
    if _NC is None:
        _NC = _build()
    in_maps = _make_in_maps(pred, target)
    res = run_bass_kernel_spmd(_NC, in_maps, core_ids=list(range(NCORES)))
    fwd = np.concatenate([res.results[m]["out"] for m in range(NFWD)]
                         ).astype(np.float64)
    bwd = np.concatenate([res.results[NFWD + m]["out"] for m in range(NFWD)]
                         ).astype(np.float64)
    vals = _combine(fwd, bwd)
    return np.float32(vals.mean(dtype=np.float64))
